# revision 11
# baseline (speedup 1.0000x reference)
"""Trainium2 Bass kernel for nn_DR_CML (data-parallel over batch, 8 cores).

Math: xm[b,i,j] = x[b,i]*lm_w[j] + lm_b[j] means every row of `loo` is a
linear function of the scalar s[b,i] = xbar[b] - x[b,i]/xd.  The tiny
H=7 MLPs applied to loo therefore collapse to scalar piecewise-linear
functions of s (weights folded into per-unit ACT scale/bias immediates),
and sum_i over the [B,K,xd-1] diff tensor collapses to a quadratic in y
with per-row coefficients:
    S[b,k] = sum_i diff[b,k,i]
           = (xd-1)*positive[b,k] + 0.5*(A[b] - 2*B[b]*y + C[b]*y^2 + D[b])
with A = sum_i e^{-lvn} mun^2, B = sum_i e^{-lvn} mun, C = sum_i e^{-lvn},
D = sum_i lvn.  Each core reduces its 64 batch rows to a [4,33] tile of
masked partial sums via one PE matmul; the host sums 8 tiles and applies
the final ~200-flop formula.
"""
import math

import numpy as np

B, XD, K, H = 512, 512, 32, 7
NCORES = 8
BL = B // NCORES  # 64 rows per core

_prog_cache = {}


def _fold_consts(p):
    """Fold linear_map + MLP weights into scalar-MLP coefficients (f64)."""
    lm_w = p['lm_w'].astype(np.float64)
    lm_b = p['lm_b'].astype(np.float64)
    c = lm_b * (XD - 1) / XD

    def fold(w1, b1):
        u = lm_w @ w1.astype(np.float64)            # [H]
        v_base = lm_b @ w1.astype(np.float64) + b1.astype(np.float64)
        v_c = c @ w1.astype(np.float64) + b1.astype(np.float64)
        return u, v_base, v_c

    u_mu, vb_mu, vc_mu = fold(p['mu_w1'], p['mu_b1'])
    u_lv, vb_lv, vc_lv = fold(p['lv_w1'], p['lv_b1'])
    u_mun, _, vc_mun = fold(p['mun_w1'], p['mun_b1'])
    u_lvn, _, vc_lvn = fold(p['lvn_w1'], p['lvn_b1'])

    out = {
        'u_mu': u_mu, 'vb_mu': vb_mu, 'vc_mu': vc_mu,
        'u_lv': u_lv, 'vb_lv': vb_lv, 'vc_lv': vc_lv,
        'u_mun': u_mun, 'vc_mun': vc_mun,
        'u_lvn': u_lvn, 'vc_lvn': vc_lvn,
        'w2_mu': p['mu_w2'][:, 0].astype(np.float64),
        'w2_lv': p['lv_w2'][:, 0].astype(np.float64),
        'w2_mun': p['mun_w2'][:, 0].astype(np.float64),
        'w2_lvn': p['lvn_w2'][:, 0].astype(np.float64),
        'b2_mu': float(p['mu_b2'][0]), 'b2_lv': float(p['lv_b2'][0]),
        'b2_mun': float(p['mun_b2'][0]), 'b2_lvn': float(p['lvn_b2'][0]),
        'ps_b': float(p['ps_b'][0]),
    }
    out['units_lvn'] = [(float(abs(w2) * u), float(abs(w2) * v), float(np.sign(w2)))
                        for u, v, w2 in zip(out['u_lvn'], out['vc_lvn'], out['w2_lvn'])]
    out['units_mun'] = [(float(abs(w2) * u), float(abs(w2) * v), float(np.sign(w2)))
                        for u, v, w2 in zip(out['u_mun'], out['vc_mun'], out['w2_mun'])]
    bias_vals = []
    for a, c, sgn in out['units_lvn'] + out['units_mun']:
        if sgn != 0:
            bias_vals.append(c)
    bias_vals += [out['b2_lvn'], out['b2_lv'], -math.log(2.0), -out['ps_b']]
    seen, ordered = set(), []
    for v in bias_vals:
        if v not in seen:
            seen.add(v)
            ordered.append(v)
    out['bias_vals'] = ordered
    return out


def _emit_hidden_combine(nc, pool, s_ap, units, tag, bc):
    """Emit relu hidden units + signed combine for one scalar MLP.

    units: list of (a, c, sign) with |w2| folded into a, c.
    bc(val): per-partition constant AP for an activation bias.
    Returns (acc_tile, sgn): acc = sgn * sum_h w2_h*relu(u_h*s+v_h).
    """
    import concourse.mybir as mybir
    Act = mybir.ActivationFunctionType
    Alu = mybir.AluOpType
    f32 = mybir.dt.float32

    pos_tiles, neg_tiles = [], []
    for idx, (a, c, sgn) in enumerate(units):
        if sgn == 0:
            continue
        t = pool.tile([BL, XD], f32, tag=f"{tag}_h{idx}")
        nc.scalar.activation(t[:], s_ap, Act.Relu, bias=bc(c), scale=float(a))
        (pos_tiles if sgn > 0 else neg_tiles).append(t)

    def acc_list(tiles, tag2):
        if len(tiles) == 1:
            return tiles[0]
        acc = pool.tile([BL, XD], f32, tag=f"{tag2}_0")
        nc.vector.tensor_tensor(acc[:], tiles[0][:], tiles[1][:], Alu.add)
        for i, t in enumerate(tiles[2:]):
            acc2 = pool.tile([BL, XD], f32, tag=f"{tag2}_{i + 1}")
            nc.vector.tensor_tensor(acc2[:], acc[:], t[:], Alu.add)
            acc = acc2
        return acc

    if pos_tiles and neg_tiles:
        ap_ = acc_list(pos_tiles, f"{tag}_ap")
        an_ = acc_list(neg_tiles, f"{tag}_an")
        acc = pool.tile([BL, XD], f32, tag=f"{tag}_acc")
        nc.vector.tensor_tensor(acc[:], ap_[:], an_[:], Alu.subtract)
        return acc, 1.0
    if pos_tiles:
        return acc_list(pos_tiles, f"{tag}_acc"), 1.0
    if neg_tiles:
        return acc_list(neg_tiles, f"{tag}_acc"), -1.0
    z = pool.tile([BL, XD], f32, tag=f"{tag}_acc")
    nc.vector.memset(z[:], 0.0)
    return z, 1.0


def _build_program(fc):
    """Build + compile the per-core Bass program. fc: folded consts."""
    from contextlib import ExitStack
    import concourse.bass as bass
    import concourse.tile as tile
    from concourse import bacc, mybir

    f32 = mybir.dt.float32
    Alu = mybir.AluOpType
    Act = mybir.ActivationFunctionType
    N1 = XD - 1  # 511

    nc = bacc.Bacc("TRN2", target_bir_lowering=False, debug=False,
                   num_devices=NCORES)

    x_d = nc.dram_tensor("x", [BL, XD], f32, kind="ExternalInput").ap()
    y_d = nc.dram_tensor("y", [BL, K], f32, kind="ExternalInput").ap()
    psw_d = nc.dram_tensor("psw", [BL, N1], f32, kind="ExternalInput").ap()
    posw_d = nc.dram_tensor("posw", [4 * H, 2], f32, kind="ExternalInput").ap()
    w2sel_d = nc.dram_tensor("w2sel", [4 * H, 4], f32, kind="ExternalInput").ap()
    iden_d = nc.dram_tensor("iden", [BL, BL], f32, kind="ExternalInput").ap()
    NB = len(fc['bias_vals'])
    bconst_d = nc.dram_tensor("bconst", [BL, NB], f32, kind="ExternalInput").ap()
    out_d = nc.dram_tensor("out", [4, K + 1], f32, kind="ExternalOutput").ap()

    b2_mun, b2_lvn = fc['b2_mun'], fc['b2_lvn']
    b2_mu, b2_lv = fc['b2_mu'], fc['b2_lv']
    LN2 = math.log(2.0)

    with tile.TileContext(nc) as tc, ExitStack() as ctx:
        sb = ctx.enter_context(tc.tile_pool(name="sb", bufs=1))
        ps = ctx.enter_context(tc.tile_pool(name="ps", bufs=1, space="PSUM"))

        # ---- DMA inputs
        tx = sb.tile([BL, XD], f32, tag="tx")
        nc.sync.dma_start(tx[:], x_d)
        ty = sb.tile([BL, K], f32, tag="ty")
        nc.sync.dma_start(ty[:], y_d)
        tpsw = sb.tile([BL, N1], f32, tag="tpsw")
        nc.sync.dma_start(tpsw[:], psw_d)
        tposw = sb.tile([4 * H, 2], f32, tag="tposw")
        nc.sync.dma_start(tposw[:], posw_d)
        tw2sel = sb.tile([4 * H, 4], f32, tag="tw2sel")
        nc.sync.dma_start(tw2sel[:], w2sel_d)
        tiden = sb.tile([BL, BL], f32, tag="tiden")
        nc.sync.dma_start(tiden[:], iden_d)
        tbc = sb.tile([BL, NB], f32, tag="tbc")
        nc.sync.dma_start(tbc[:], bconst_d)
        bias_idx = {v: i for i, v in enumerate(fc['bias_vals'])}

        def bc(val, p0=0, p1=BL):
            j = bias_idx[val]
            return tbc[p0:p1, j:j + 1]

        # ---- stage A: xbar, s
        xsum = sb.tile([BL, 1], f32, tag="xsum")
        nc.vector.tensor_reduce(xsum[:], tx[:], mybir.AxisListType.X, Alu.add)
        xbar = sb.tile([BL, 1], f32, tag="xbar")
        nc.vector.tensor_scalar(xbar[:], xsum[:], 1.0 / XD, None, Alu.mult)
        s = sb.tile([BL, XD], f32, tag="s")
        nc.vector.tensor_scalar(s[:], tx[:], -1.0 / XD, xbar[:], Alu.mult, Alu.add)

        # ---- hidden units + combines (lvn first: tanh/exp consume it)
        lacc, sgn_l = _emit_hidden_combine(nc, sb, s[:], fc['units_lvn'], "lv", bc)
        macc, sgn_m = _emit_hidden_combine(nc, sb, s[:], fc['units_mun'], "mn", bc)

        # lvn = tanh(sgn_l*lacc + b2_lvn), D = sum_i lvn  (i < 511 only)
        lvn = sb.tile([BL, N1], f32, tag="lvn")
        Dv = sb.tile([BL, 1], f32, tag="Dv")
        nc.scalar.activation(lvn[:], lacc[:, 0:N1], Act.Tanh,
                             bias=bc(b2_lvn), scale=float(sgn_l), accum_out=Dv[:])
        # e = exp(-lvn), C = sum e
        ev = sb.tile([BL, N1], f32, tag="ev")
        Cv = sb.tile([BL, 1], f32, tag="Cv")
        nc.scalar.activation(ev[:], lvn[:], Act.Exp, bias=0.0, scale=-1.0,
                             accum_out=Cv[:])
        # em = e*macc (accum Bt), emm = em*macc (accum At)
        em = sb.tile([BL, N1], f32, tag="em")
        Bt = sb.tile([BL, 1], f32, tag="Bt")
        nc.vector.scalar_tensor_tensor(em[:], ev[:], 1.0, macc[:, 0:N1],
                                       Alu.mult, Alu.mult, accum_out=Bt[:])
        emm = sb.tile([BL, N1], f32, tag="emm")
        At = sb.tile([BL, 1], f32, tag="At")
        nc.vector.scalar_tensor_tensor(emm[:], em[:], 1.0, macc[:, 0:N1],
                                       Alu.mult, Alu.mult, accum_out=At[:])

        # ---- fixups: quadratic coeffs  (mun = sgn_m*macc + b2_mun)
        # q2 = 0.5*C ; q1 = -(sgn_m*Bt + b2*C) ; q0 = 0.5*(At + D) + b2*sgn_m*Bt + 0.5*b2^2*C
        q2 = sb.tile([BL, 1], f32, tag="q2")
        nc.vector.tensor_scalar(q2[:], Cv[:], 0.5, None, Alu.mult)
        bC = sb.tile([BL, 1], f32, tag="bC")
        nc.vector.tensor_scalar(bC[:], Cv[:], -b2_mun, None, Alu.mult)
        q1 = sb.tile([BL, 1], f32, tag="q1")
        nc.vector.scalar_tensor_tensor(q1[:], Bt[:], -float(sgn_m), bC[:],
                                       Alu.mult, Alu.add)
        w1_ = sb.tile([BL, 1], f32, tag="w1_")
        nc.vector.tensor_tensor(w1_[:], At[:], Dv[:], Alu.add)
        w2_ = sb.tile([BL, 1], f32, tag="w2_")
        nc.vector.tensor_scalar(w2_[:], w1_[:], 0.5, None, Alu.mult)
        w3_ = sb.tile([BL, 1], f32, tag="w3_")
        nc.vector.scalar_tensor_tensor(w3_[:], Bt[:], float(sgn_m) * b2_mun,
                                       w2_[:], Alu.mult, Alu.add)
        q0 = sb.tile([BL, 1], f32, tag="q0")
        nc.vector.scalar_tensor_tensor(q0[:], Cv[:], 0.5 * b2_mun * b2_mun,
                                       w3_[:], Alu.mult, Alu.add)

        # ---- positive branch: mu0/lv0 (from xbar), mu1/lv1 (from s[:,510])
        xz = sb.tile([BL, 4 * H], f32, tag="xz")
        xb_b = xbar[:, 0:1].broadcast_to([BL, H])
        sl_b = s[:, XD - 2:XD - 1].broadcast_to([BL, H])
        nc.vector.tensor_copy(xz[:, 0:H], xb_b)
        nc.vector.tensor_copy(xz[:, H:2 * H], sl_b)
        nc.vector.tensor_copy(xz[:, 2 * H:3 * H], xb_b)
        nc.vector.tensor_copy(xz[:, 3 * H:4 * H], sl_b)

        zt = ps.tile([4 * H, BL], f32, tag="zt")
        nc.tensor.transpose(zt[:], xz[:], tiden[:])
        hpos = sb.tile([4 * H, BL], f32, tag="hpos")
        nc.scalar.activation(hpos[:], zt[:], Act.Relu,
                             bias=tposw[:, 1:2], scale=tposw[:, 0:1])
        out4 = ps.tile([4, BL], f32, tag="out4")
        nc.tensor.matmul(out4[:], tw2sel[:], hpos[:], start=True, stop=True)
        m4 = sb.tile([4, BL], f32, tag="m4")
        nc.vector.tensor_copy(m4[:], out4[:])
        tm4 = ps.tile([BL, 4], f32, tag="tm4")
        nc.tensor.transpose(tm4[:], m4[:], tiden[0:4, 0:4])
        mlv = sb.tile([BL, 4], f32, tag="mlv")
        nc.vector.tensor_scalar(mlv[:, 0:2], tm4[:, 0:2], b2_mu, None, Alu.add)
        nc.scalar.activation(mlv[:, 2:4], tm4[:, 2:4], Act.Tanh, bias=bc(b2_lv))
        mu0, mu1 = mlv[:, 0:1], mlv[:, 1:2]
        lv0, lv1 = mlv[:, 2:3], mlv[:, 3:4]

        # g = -0.5*exp(-lv), kk = -lv/2 per branch
        g1e = sb.tile([BL, 1], f32, tag="g1e")
        nc.scalar.activation(g1e[:], lv1, Act.Exp, bias=bc(-LN2), scale=-1.0)
        g1n = sb.tile([BL, 1], f32, tag="g1n")
        nc.vector.tensor_scalar(g1n[:], g1e[:], -1.0, None, Alu.mult)
        k1 = sb.tile([BL, 1], f32, tag="k1")
        nc.vector.tensor_scalar(k1[:], lv1, -0.5, None, Alu.mult)
        g0e = sb.tile([BL, 1], f32, tag="g0e")
        nc.scalar.activation(g0e[:], lv0, Act.Exp, bias=bc(-LN2), scale=-1.0)
        g0n = sb.tile([BL, 1], f32, tag="g0n")
        nc.vector.tensor_scalar(g0n[:], g0e[:], -1.0, None, Alu.mult)
        k0 = sb.tile([BL, 1], f32, tag="k0")
        nc.vector.tensor_scalar(k0[:], lv0, -0.5, None, Alu.mult)

        # pos[b,k]: k=0 from (mu0,lv0), k>=1 from (mu1,lv1)
        d1 = sb.tile([BL, K], f32, tag="d1")
        nc.vector.tensor_scalar(d1[:], ty[:], mu1, None, Alu.subtract)
        d1sq = sb.tile([BL, K], f32, tag="d1sq")
        nc.vector.tensor_tensor(d1sq[:], d1[:], d1[:], Alu.mult)
        pos = sb.tile([BL, K], f32, tag="pos")
        nc.vector.tensor_scalar(pos[:], d1sq[:], g1n[:], k1[:], Alu.mult, Alu.add)
        d0 = sb.tile([BL, 1], f32, tag="d0")
        nc.vector.tensor_scalar(d0[:], ty[:, 0:1], mu0, None, Alu.subtract)
        d0sq = sb.tile([BL, 1], f32, tag="d0sq")
        nc.vector.tensor_tensor(d0sq[:], d0[:], d0[:], Alu.mult)
        nc.vector.tensor_scalar(pos[:, 0:1], d0sq[:], g0n[:], k0[:],
                                Alu.mult, Alu.add)

        # ---- propensity weights + masks
        junk = sb.tile([BL, N1], f32, tag="junk")
        zp = sb.tile([BL, 1], f32, tag="zp")
        nc.vector.scalar_tensor_tensor(junk[:], tx[:, 0:N1], 1.0, tpsw[:],
                                       Alu.mult, Alu.mult, accum_out=zp[:])
        epr = sb.tile([BL, 1], f32, tag="epr")
        nc.scalar.activation(epr[:], zp[:], Act.Exp, bias=bc(-fc['ps_b']), scale=-1.0)
        opp = sb.tile([BL, 1], f32, tag="opp")
        nc.vector.tensor_scalar(opp[:], epr[:], 1.0, None, Alu.add)
        prop = sb.tile([BL, 1], f32, tag="prop")
        nc.vector.reciprocal(prop[:], opp[:])
        pw1 = sb.tile([BL, 1], f32, tag="pw1")
        nc.vector.tensor_scalar(pw1[:], prop[:], 1e-4, None, Alu.add)
        w1v = sb.tile([BL, 1], f32, tag="w1v")
        nc.vector.reciprocal(w1v[:], pw1[:])
        pw0 = sb.tile([BL, 1], f32, tag="pw0")
        nc.vector.tensor_scalar(pw0[:], prop[:], -1.0, 1.0 + 1e-4,
                                Alu.mult, Alu.add)
        w0v = sb.tile([BL, 1], f32, tag="w0v")
        nc.vector.reciprocal(w0v[:], pw0[:])

        F = sb.tile([BL, 4], f32, tag="F")
        nc.vector.tensor_scalar(F[:, 0:1], tx[:, XD - 1:XD], 0.0, None,
                                Alu.is_equal)
        nc.vector.tensor_scalar(F[:, 2:3], tx[:, XD - 1:XD], 1.0, None,
                                Alu.is_equal)
        nc.vector.tensor_tensor(F[:, 1:2], F[:, 0:1], w0v[:], Alu.mult)
        nc.vector.tensor_tensor(F[:, 3:4], F[:, 2:3], w1v[:], Alu.mult)

        # ---- S = 511*pos + q0 + q1*y + q2*y^2 ; R = [S | 1]
        R = sb.tile([BL, K + 1], f32, tag="R")
        yt2 = sb.tile([BL, K], f32, tag="yt2")
        nc.vector.tensor_tensor(yt2[:], ty[:], ty[:], Alu.mult)
        S1 = sb.tile([BL, K], f32, tag="S1")
        nc.vector.tensor_scalar(S1[:], yt2[:], q2[:], q0[:], Alu.mult, Alu.add)
        S2 = sb.tile([BL, K], f32, tag="S2")
        nc.vector.scalar_tensor_tensor(S2[:], ty[:], q1[:], S1[:],
                                       Alu.mult, Alu.add)
        nc.vector.scalar_tensor_tensor(R[:, 0:K], pos[:], float(XD - 1), S2[:],
                                       Alu.mult, Alu.add)
        nc.vector.memset(R[:, K:K + 1], 1.0)

        # ---- masked partial sums: out[4, 33] = F.T @ R
        P = ps.tile([4, K + 1], f32, tag="P")
        nc.tensor.matmul(P[:], F[:], R[:], start=True, stop=True)
        outs = sb.tile([4, K + 1], f32, tag="outs")
        nc.vector.tensor_copy(outs[:], P[:])
        nc.sync.dma_start(out_d, outs[:])

    nc.compile()
    return nc


def _host_inputs(inputs, fc):
    """Per-core in_maps + shared const tensors."""
    x = np.ascontiguousarray(inputs['x_samples'], dtype=np.float32)
    y = np.ascontiguousarray(inputs['y_samples'], dtype=np.float32)
    ps_w = inputs['ps_w'].astype(np.float32)[:, 0]           # [511]
    psw_rep = np.ascontiguousarray(np.broadcast_to(ps_w, (BL, XD - 1)))

    posw = np.zeros((4 * H, 2), np.float32)
    posw[0:H, 0] = fc['u_mu']; posw[0:H, 1] = fc['vb_mu']
    posw[H:2 * H, 0] = fc['u_mu']; posw[H:2 * H, 1] = fc['vc_mu']
    posw[2 * H:3 * H, 0] = fc['u_lv']; posw[2 * H:3 * H, 1] = fc['vb_lv']
    posw[3 * H:4 * H, 0] = fc['u_lv']; posw[3 * H:4 * H, 1] = fc['vc_lv']

    w2sel = np.zeros((4 * H, 4), np.float32)
    w2sel[0:H, 0] = fc['w2_mu']
    w2sel[H:2 * H, 1] = fc['w2_mu']
    w2sel[2 * H:3 * H, 2] = fc['w2_lv']
    w2sel[3 * H:4 * H, 3] = fc['w2_lv']

    iden = np.eye(BL, dtype=np.float32)
    bconst = np.ascontiguousarray(
        np.broadcast_to(np.array(fc['bias_vals'], np.float32), (BL, len(fc['bias_vals']))))

    in_maps = []
    for i in range(NCORES):
        in_maps.append({
            'x': x[i * BL:(i + 1) * BL],
            'y': y[i * BL:(i + 1) * BL],
            'psw': psw_rep, 'posw': posw, 'w2sel': w2sel, 'iden': iden,
            'bconst': bconst,
        })
    return in_maps


def _combine(parts):
    """Host-side final combine of the 8 [4,33] partial tiles."""
    tot = np.zeros((4, K + 1), np.float64)
    for p in parts:
        tot += p.astype(np.float64)
    P0, n0 = tot[0, :K], tot[0, K]
    Q0, r0 = tot[1, :K], tot[1, K]
    P1, n1 = tot[2, :K], tot[2, K]
    Q1, r1 = tot[3, :K], tot[3, K]
    d0 = n0 * (XD - 1)
    d1 = n1 * (XD - 1)
    cmi0 = P0 / d0
    cmi1 = P1 / d1
    dr = 0.5 * ((XD - 1) * cmi0 * (n0 - r0) + Q0) / d0 \
       + 0.5 * ((XD - 1) * cmi1 * (n1 - r1) + Q1) / d1
    cmi_dims = (np.abs(cmi0 + cmi1) / 2.0).astype(np.float32)
    drs = np.abs(dr).astype(np.float32)
    return cmi_dims, drs


def _param_key(inputs):
    import hashlib
    hsh = hashlib.sha256()
    for k in sorted(inputs):
        if k in ('x_samples', 'y_samples'):
            continue
        hsh.update(k.encode())
        hsh.update(np.ascontiguousarray(inputs[k]).tobytes())
    return hsh.hexdigest()


def kernel(**inputs):
    from concourse.bass_utils import run_bass_kernel_spmd

    fc = _fold_consts(inputs)
    key = _param_key(inputs)
    if key not in _prog_cache:
        _prog_cache[key] = _build_program(fc)
    nc = _prog_cache[key]

    in_maps = _host_inputs(inputs, fc)
    res = run_bass_kernel_spmd(nc, in_maps, core_ids=list(range(NCORES)))
    parts = [r['out'] for r in res.results]
    return _combine(parts)


# revision 16
# speedup vs baseline: 1.3436x; 1.3436x over previous
"""Trainium2 Bass kernel for nn_DR_CML (data-parallel over batch, 8 cores).

Math: xm[b,i,j] = x[b,i]*lm_w[j] + lm_b[j] means every row of `loo` is a
linear function of the scalar s[b,i] = xbar[b] - x[b,i]/xd.  The tiny
H=7 MLPs applied to loo collapse to scalar piecewise-linear functions of
s, and sum_i over the [B,K,xd-1] diff tensor collapses to a quadratic in
y with per-row coefficients:
    S[b,k] = (xd-1)*positive[b,k] + 0.5*(A[b] - 2*B[b]*y + C[b]*y^2 + D[b])
with A = sum_i e^{-lvn} mun^2, B = sum_i e^{-lvn} mun, C = sum_i e^{-lvn},
D = sum_i lvn (i < xd-1).

Per-call exact specialization: the s range is tiny (xbar ~ N(0,1/xd)), so
most relu units are affine-or-zero over the actual data; they fold into
one exact linear term.  Only units whose breakpoint the data straddles
are emitted as ACT relus.

Layout: x is repacked [2*(B/8), xd/2] = [128, 256] so every big op uses
all 128 partitions; per-row sums are halved per partition and pair-summed
with one PE matmul against a constant M (M[k,m]=1 iff k==m or k==m^64).
Each core reduces its 64 rows to a [4,33] tile of masked partial sums
(one PE matmul); the host sums 8 tiles and applies the final formula.
"""
import math

import numpy as np

B, XD, K, H = 512, 512, 32, 7
NCORES = 8
BL = B // NCORES          # 64 rows per core
HC = XD // 2              # 256 columns after repack
N1 = XD - 1

_prog_cache = {}


def _fold_consts(p):
    """Fold linear_map + MLP weights into scalar-MLP coefficients (f64)."""
    lm_w = p['lm_w'].astype(np.float64)
    lm_b = p['lm_b'].astype(np.float64)
    c = lm_b * (XD - 1) / XD

    def fold(w1, b1):
        u = lm_w @ w1.astype(np.float64)
        v_base = lm_b @ w1.astype(np.float64) + b1.astype(np.float64)
        v_c = c @ w1.astype(np.float64) + b1.astype(np.float64)
        return u, v_base, v_c

    u_mu, vb_mu, vc_mu = fold(p['mu_w1'], p['mu_b1'])
    u_lv, vb_lv, vc_lv = fold(p['lv_w1'], p['lv_b1'])
    u_mun, _, vc_mun = fold(p['mun_w1'], p['mun_b1'])
    u_lvn, _, vc_lvn = fold(p['lvn_w1'], p['lvn_b1'])

    return {
        'u_mu': u_mu, 'vb_mu': vb_mu, 'vc_mu': vc_mu,
        'u_lv': u_lv, 'vb_lv': vb_lv, 'vc_lv': vc_lv,
        'u_mun': u_mun, 'vc_mun': vc_mun,
        'u_lvn': u_lvn, 'vc_lvn': vc_lvn,
        'w2_mu': p['mu_w2'][:, 0].astype(np.float64),
        'w2_lv': p['lv_w2'][:, 0].astype(np.float64),
        'w2_mun': p['mun_w2'][:, 0].astype(np.float64),
        'w2_lvn': p['lvn_w2'][:, 0].astype(np.float64),
        'b2_mu': float(p['mu_b2'][0]), 'b2_lv': float(p['lv_b2'][0]),
        'b2_mun': float(p['mun_b2'][0]), 'b2_lvn': float(p['lvn_b2'][0]),
        'ps_b': float(p['ps_b'][0]),
    }


def _specialize(fc, x):
    """Exact per-call relu pruning over the data's s range (i <= xd-2).

    Returns {'mun': (alpha, beta, active), 'lvn': (...)} where active is a
    list of (a, c, sgn) relu units (|w2| folded in) and alpha includes the
    relevant output bias (b2_mun / b2_lvn).
    """
    x64 = x.astype(np.float64)
    xbar = x64.mean(1)
    s = xbar[:, None] - x64[:, :N1] / XD
    smin, smax = float(s.min()), float(s.max())

    out = {}
    for name, u_all, v_all, w2_all, b2 in (
            ('mun', fc['u_mun'], fc['vc_mun'], fc['w2_mun'], fc['b2_mun']),
            ('lvn', fc['u_lvn'], fc['vc_lvn'], fc['w2_lvn'], fc['b2_lvn'])):
        alpha, beta = b2, 0.0
        active = []
        for u, v, w2 in zip(u_all, v_all, w2_all):
            if w2 == 0.0:
                continue
            lo = min(u * smin, u * smax) + v
            hi = max(u * smin, u * smax) + v
            if lo >= 0.0:          # linear over the data
                alpha += w2 * v
                beta += w2 * u
            elif hi <= 0.0:        # identically zero over the data
                pass
            else:                  # genuinely piecewise on the data
                active.append((float(abs(w2) * u), float(abs(w2) * v),
                               1.0 if w2 > 0 else -1.0))
        out[name] = (float(alpha), float(beta), active)
    return out


def _const_layout(fc, spec):
    """Column layout of the packed consts tensor + bias-value table."""
    bias_vals = []
    for _, _, act in (spec['mun'], spec['lvn']):
        for a, c, sgn in act:
            bias_vals.append(c)
    bias_vals += [fc['b2_mu'], fc['b2_lv'], -math.log(2.0), -fc['ps_b']]
    seen, ordered = set(), []
    for v in bias_vals:
        if v not in seen:
            seen.add(v)
            ordered.append(v)
    nb = len(ordered)
    lay = {
        'bias_vals': ordered,
        'bias0': 128,                 # after M
        'posa': 128 + nb,
        'posc': 128 + nb + 1,
        'w2sel': 128 + nb + 2,        # 4 cols
        'width': 128 + nb + 6,
    }
    return lay


def _build_program(fc, spec, lay):
    from contextlib import ExitStack
    import concourse.tile as tile
    from concourse import bacc, mybir

    f32 = mybir.dt.float32
    bf16 = mybir.dt.bfloat16
    Alu = mybir.AluOpType
    Act = mybir.ActivationFunctionType

    nc = bacc.Bacc("TRN2", target_bir_lowering=False, debug=False,
                   num_devices=NCORES)

    xt_d = nc.dram_tensor("xt", [2 * BL, HC], f32, kind="ExternalInput").ap()
    yv_d = nc.dram_tensor("yv", [BL, K], f32, kind="ExternalInput").ap()
    psw_d = nc.dram_tensor("psw", [2 * BL, HC], f32, kind="ExternalInput").ap()
    tc_d = nc.dram_tensor("consts", [128, lay['width']], f32,
                          kind="ExternalInput").ap()
    out_d = nc.dram_tensor("out", [4, K + 1], f32, kind="ExternalOutput").ap()

    bias_idx = {v: lay['bias0'] + i for i, v in enumerate(lay['bias_vals'])}
    LN2 = math.log(2.0)
    a_mun, b_mun, act_mun = spec['mun']
    a_lvn, b_lvn, act_lvn = spec['lvn']

    with tile.TileContext(nc) as tcx, ExitStack() as ctx:
        sb = ctx.enter_context(tcx.tile_pool(name="sb", bufs=1))
        ps = ctx.enter_context(tcx.tile_pool(name="ps", bufs=1, space="PSUM"))

        # ---- DMAs: x + consts on sync; y + psw on gpsimd queue
        tx = sb.tile([128, HC], f32, tag="tx")
        nc.sync.dma_start(tx[:], xt_d)
        tc = sb.tile([128, lay['width']], f32, tag="tc")
        nc.sync.dma_start(tc[:], tc_d)
        ty = sb.tile([BL, K], f32, tag="ty")
        nc.gpsimd.dma_start(ty[:], yv_d)
        tpsw = sb.tile([128, HC], f32, tag="tpsw")
        nc.gpsimd.dma_start(tpsw[:], psw_d)

        def bc(val, p0=0, p1=128):
            j = bias_idx[val]
            return tc[p0:p1, j:j + 1]

        M = tc[:, 0:128]
        Mhalf = tc[:, 0:BL]
        iden = tc[0:BL, 0:BL]

        # ---- xbar, s (f32, [128, 256])
        xsum = sb.tile([128, 1], f32, tag="xsum")
        nc.vector.tensor_reduce(xsum[:], tx[:], mybir.AxisListType.X, Alu.add)
        xbs = ps.tile([128, 1], f32, tag="xbs")
        nc.tensor.matmul(xbs[:], M, xsum[:], start=True, stop=True)
        xbar = sb.tile([128, 1], f32, tag="xbar")
        nc.vector.tensor_scalar(xbar[:], xbs[:], 1.0 / XD, None, Alu.mult)
        s = sb.tile([128, HC], f32, tag="s")
        nc.vector.tensor_scalar(s[:], tx[:], -1.0 / XD, xbar[:], Alu.mult, Alu.add)

        # ---- mun tile: affine + active relus (bf16)
        def build_pwl(alpha, beta, active, tag):
            aff = sb.tile([128, HC], f32, tag=f"{tag}_aff")
            nc.vector.tensor_scalar(aff[:], s[:], beta, alpha, Alu.mult, Alu.add)
            cur = aff
            for idx, (a, c, sgn) in enumerate(active):
                t = sb.tile([128, HC], f32, tag=f"{tag}_u{idx}")
                nc.scalar.activation(t[:], s[:], Act.Relu, bias=bc(c),
                                     scale=float(a))
                nxt = sb.tile([128, HC], f32, tag=f"{tag}_c{idx}")
                nc.vector.tensor_tensor(nxt[:], cur[:], t[:],
                                        Alu.add if sgn > 0 else Alu.subtract)
                cur = nxt
            return cur

        mun_t = build_pwl(a_mun, b_mun, act_mun, "mn")
        lva = build_pwl(a_lvn, b_lvn, act_lvn, "lv")

        # ---- lvn = tanh(lva), ev = exp(-lvn); zero the (i=xd-1) slot, then
        #      accumulate A,B,C,D into columns of one acc tile.
        lvn = sb.tile([128, HC], f32, tag="lvn")
        nc.scalar.activation(lvn[:], lva[:], Act.Tanh, bias=0.0, scale=1.0)
        ev = sb.tile([128, HC], f32, tag="ev")
        nc.scalar.activation(ev[:], lvn[:], Act.Exp, bias=0.0, scale=-1.0)

        acc = sb.tile([128, 5], f32, tag="acc")   # cols: A,B,C,D,zp
        nc.vector.memset(lvn[BL:128, HC - 1:HC], 0.0)
        nc.vector.tensor_reduce(acc[:, 3:4], lvn[:], mybir.AxisListType.X,
                                Alu.add)
        nc.vector.memset(ev[BL:128, HC - 1:HC], 0.0)
        nc.vector.tensor_reduce(acc[:, 2:3], ev[:], mybir.AxisListType.X,
                                Alu.add)
        em = sb.tile([128, HC], f32, tag="em")
        nc.vector.scalar_tensor_tensor(em[:], ev[:], 1.0, mun_t[:],
                                       Alu.mult, Alu.mult,
                                       accum_out=acc[:, 1:2])
        emm = sb.tile([128, HC], f32, tag="emm")
        nc.vector.scalar_tensor_tensor(emm[:], em[:], 1.0, mun_t[:],
                                       Alu.mult, Alu.mult,
                                       accum_out=acc[:, 0:1])
        junk = sb.tile([128, HC], f32, tag="junk")
        nc.vector.scalar_tensor_tensor(junk[:], tx[:], 1.0, tpsw[:],
                                       Alu.mult, Alu.mult,
                                       accum_out=acc[:, 4:5])

        # ---- staging for s[.,xd-2] and x[.,xd-1] extraction (rows 64:128)
        stg = sb.tile([128, 2], f32, tag="stg")
        nc.gpsimd.memset(stg[0:BL, :], 0.0)
        nc.gpsimd.tensor_copy(stg[BL:128, 0:1], s[BL:128, HC - 2:HC - 1])
        nc.gpsimd.tensor_copy(stg[BL:128, 1:2], tx[BL:128, HC - 1:HC])
        sel = ps.tile([BL, 2], f32, tag="sel")
        nc.tensor.matmul(sel[:], Mhalf, stg[:], start=True, stop=True)
        selsb = sb.tile([BL, 2], f32, tag="selsb")
        nc.vector.tensor_copy(selsb[:], sel[:])

        accp = ps.tile([BL, 5], f32, tag="accp")
        nc.tensor.matmul(accp[:], Mhalf, acc[:], start=True, stop=True)
        accs = sb.tile([BL, 5], f32, tag="accs")
        nc.vector.tensor_copy(accs[:], accp[:])

        # ---- positive branch: mu0/lv0 (xbar), mu1/lv1 (s[.,xd-2])
        xz = sb.tile([BL, 4 * H], f32, tag="xz")
        xb_b = xbar[0:BL, 0:1].broadcast_to([BL, H])
        sl_b = selsb[:, 0:1].broadcast_to([BL, H])
        nc.gpsimd.tensor_copy(xz[:, 0:H], xb_b)
        nc.gpsimd.tensor_copy(xz[:, H:2 * H], sl_b)
        nc.gpsimd.tensor_copy(xz[:, 2 * H:3 * H], xb_b)
        nc.gpsimd.tensor_copy(xz[:, 3 * H:4 * H], sl_b)
        zt = ps.tile([4 * H, BL], f32, tag="zt")
        nc.tensor.transpose(zt[:], xz[:], iden)
        hpos = sb.tile([4 * H, BL], f32, tag="hpos")
        nc.scalar.activation(hpos[:], zt[:], Act.Relu,
                             bias=tc[0:4 * H, lay['posc']:lay['posc'] + 1],
                             scale=tc[0:4 * H, lay['posa']:lay['posa'] + 1])
        mlvp = ps.tile([BL, 4], f32, tag="mlvp")
        nc.tensor.matmul(mlvp[:], hpos[:],
                         tc[0:4 * H, lay['w2sel']:lay['w2sel'] + 4],
                         start=True, stop=True)
        mlv = sb.tile([BL, 4], f32, tag="mlv")
        nc.scalar.activation(mlv[:, 0:2], mlvp[:, 0:2], Act.Identity,
                             bias=bc(fc['b2_mu'], 0, BL), scale=1.0)
        nc.scalar.activation(mlv[:, 2:4], mlvp[:, 2:4], Act.Tanh,
                             bias=bc(fc['b2_lv'], 0, BL), scale=1.0)
        ge2 = sb.tile([BL, 2], f32, tag="ge2")
        nc.scalar.activation(ge2[:], mlv[:, 2:4], Act.Exp,
                             bias=bc(-LN2, 0, BL), scale=-1.0)
        gn2 = sb.tile([BL, 2], f32, tag="gn2")
        nc.gpsimd.tensor_scalar(gn2[:], ge2[:], -1.0, None, Alu.mult)
        kk2 = sb.tile([BL, 2], f32, tag="kk2")
        nc.gpsimd.tensor_scalar(kk2[:], mlv[:, 2:4], -0.5, None, Alu.mult)

        # pos[b,k]: k=0 from (mu0,lv0), k>=1 from (mu1,lv1)
        d1 = sb.tile([BL, K], f32, tag="d1")
        nc.gpsimd.tensor_scalar(d1[:], ty[:], mlv[:, 1:2], None, Alu.subtract)
        d1sq = sb.tile([BL, K], f32, tag="d1sq")
        nc.gpsimd.tensor_tensor(d1sq[:], d1[:], d1[:], Alu.mult)
        pos = sb.tile([BL, K], f32, tag="pos")
        nc.gpsimd.tensor_scalar(pos[:], d1sq[:], gn2[:, 1:2], kk2[:, 1:2],
                                Alu.mult, Alu.add)
        d0 = sb.tile([BL, 1], f32, tag="d0")
        nc.gpsimd.tensor_scalar(d0[:], ty[:, 0:1], mlv[:, 0:1], None,
                                Alu.subtract)
        d0sq = sb.tile([BL, 1], f32, tag="d0sq")
        nc.gpsimd.tensor_tensor(d0sq[:], d0[:], d0[:], Alu.mult)
        nc.gpsimd.tensor_scalar(pos[:, 0:1], d0sq[:], gn2[:, 0:1],
                                kk2[:, 0:1], Alu.mult, Alu.add)

        # ---- propensity weights:  p = 1/(1+e),  e = exp(-(z+ps_b))
        # w1 = (1+e)/(1.0001 + 1e-4 e) ; w0 = (1+e)/(1e-4 + 1.0001 e)
        epr = sb.tile([BL, 1], f32, tag="epr")
        nc.scalar.activation(epr[:], accs[:, 4:5], Act.Exp,
                             bias=bc(-fc['ps_b'], 0, BL), scale=-1.0)
        num = sb.tile([BL, 1], f32, tag="num")
        nc.vector.tensor_scalar(num[:], epr[:], 1.0, None, Alu.add)
        den1 = sb.tile([BL, 1], f32, tag="den1")
        nc.vector.tensor_scalar(den1[:], epr[:], 1e-4, 1.0 + 1e-4,
                                Alu.mult, Alu.add)
        den0 = sb.tile([BL, 1], f32, tag="den0")
        nc.vector.tensor_scalar(den0[:], epr[:], 1.0 + 1e-4, 1e-4,
                                Alu.mult, Alu.add)
        r1 = sb.tile([BL, 1], f32, tag="r1")
        nc.vector.reciprocal(r1[:], den1[:])
        w1v = sb.tile([BL, 1], f32, tag="w1v")
        nc.vector.tensor_tensor(w1v[:], num[:], r1[:], Alu.mult)
        r0 = sb.tile([BL, 1], f32, tag="r0")
        nc.vector.reciprocal(r0[:], den0[:])
        w0v = sb.tile([BL, 1], f32, tag="w0v")
        nc.vector.tensor_tensor(w0v[:], num[:], r0[:], Alu.mult)

        F = sb.tile([BL, 4], f32, tag="F")
        nc.gpsimd.tensor_scalar(F[:, 0:1], selsb[:, 1:2], 0.0, None,
                                Alu.is_equal)
        nc.gpsimd.tensor_scalar(F[:, 2:3], selsb[:, 1:2], 1.0, None,
                                Alu.is_equal)
        nc.gpsimd.tensor_tensor(F[:, 1:2], F[:, 0:1], w0v[:], Alu.mult)
        nc.gpsimd.tensor_tensor(F[:, 3:4], F[:, 2:3], w1v[:], Alu.mult)

        # ---- S = 511*pos + q0 + q1*y + q2*y^2 ; R = [S | 1]
        q2 = sb.tile([BL, 1], f32, tag="q2")
        nc.vector.tensor_scalar(q2[:], accs[:, 2:3], 0.5, None, Alu.mult)
        q1 = sb.tile([BL, 1], f32, tag="q1")
        nc.vector.tensor_scalar(q1[:], accs[:, 1:2], -1.0, None, Alu.mult)
        q0h = sb.tile([BL, 1], f32, tag="q0h")
        nc.vector.tensor_tensor(q0h[:], accs[:, 0:1], accs[:, 3:4], Alu.add)
        q0 = sb.tile([BL, 1], f32, tag="q0")
        nc.vector.tensor_scalar(q0[:], q0h[:], 0.5, None, Alu.mult)

        R = sb.tile([BL, K + 1], f32, tag="R")
        nc.gpsimd.memset(R[:, K:K + 1], 1.0)
        yt2 = sb.tile([BL, K], f32, tag="yt2")
        nc.vector.tensor_tensor(yt2[:], ty[:], ty[:], Alu.mult)
        S1 = sb.tile([BL, K], f32, tag="S1")
        nc.vector.tensor_scalar(S1[:], yt2[:], q2[:], q0[:], Alu.mult, Alu.add)
        S2 = sb.tile([BL, K], f32, tag="S2")
        nc.vector.scalar_tensor_tensor(S2[:], ty[:], q1[:], S1[:],
                                       Alu.mult, Alu.add)
        nc.vector.scalar_tensor_tensor(R[:, 0:K], pos[:], float(XD - 1),
                                       S2[:], Alu.mult, Alu.add)

        P = ps.tile([4, K + 1], f32, tag="P")
        nc.tensor.matmul(P[:], F[:], R[:], start=True, stop=True)
        outs = sb.tile([4, K + 1], f32, tag="outs")
        nc.vector.tensor_copy(outs[:], P[:])
        nc.sync.dma_start(out_d, outs[:])

    nc.compile()
    return nc


def _host_inputs(inputs, fc, spec, lay):
    x = np.ascontiguousarray(inputs['x_samples'], dtype=np.float32)
    y = np.ascontiguousarray(inputs['y_samples'], dtype=np.float32)
    ps_w = inputs['ps_w'].astype(np.float32)[:, 0]

    # psw repack: rows p<64 <- ps_w[0:256]; rows p>=64 <- ps_w[256:511] + 0
    psw2 = np.zeros((2, HC), np.float32)
    psw2[0] = ps_w[0:HC]
    psw2[1, 0:HC - 1] = ps_w[HC:N1]
    psw = np.ascontiguousarray(
        np.broadcast_to(psw2[:, None, :], (2, BL, HC)).reshape(128, HC))

    M = np.zeros((128, 128), np.float32)
    idx = np.arange(128)
    M[idx, idx] = 1.0
    M[idx ^ 64, idx] = 1.0

    consts = np.zeros((128, lay['width']), np.float32)
    consts[:, 0:128] = M
    for i, v in enumerate(lay['bias_vals']):
        consts[:, lay['bias0'] + i] = v
    posa = np.zeros(4 * H); posc = np.zeros(4 * H)
    posa[0:H] = fc['u_mu'];          posc[0:H] = fc['vb_mu']
    posa[H:2 * H] = fc['u_mu'];      posc[H:2 * H] = fc['vc_mu']
    posa[2 * H:3 * H] = fc['u_lv'];  posc[2 * H:3 * H] = fc['vb_lv']
    posa[3 * H:4 * H] = fc['u_lv'];  posc[3 * H:4 * H] = fc['vc_lv']
    consts[0:4 * H, lay['posa']] = posa
    consts[0:4 * H, lay['posc']] = posc
    w2sel = np.zeros((4 * H, 4), np.float32)
    w2sel[0:H, 0] = fc['w2_mu']
    w2sel[H:2 * H, 1] = fc['w2_mu']
    w2sel[2 * H:3 * H, 2] = fc['w2_lv']
    w2sel[3 * H:4 * H, 3] = fc['w2_lv']
    consts[0:4 * H, lay['w2sel']:lay['w2sel'] + 4] = w2sel

    in_maps = []
    for i in range(NCORES):
        xs = x[i * BL:(i + 1) * BL]                       # [64, 512]
        xt = np.ascontiguousarray(
            xs.reshape(BL, 2, HC).transpose(1, 0, 2).reshape(128, HC))
        in_maps.append({
            'xt': xt,
            'yv': y[i * BL:(i + 1) * BL],
            'psw': psw, 'consts': consts,
        })
    return in_maps


def _combine(parts):
    tot = np.zeros((4, K + 1), np.float64)
    for p in parts:
        tot += p.astype(np.float64)
    P0, n0 = tot[0, :K], tot[0, K]
    Q0, r0 = tot[1, :K], tot[1, K]
    P1, n1 = tot[2, :K], tot[2, K]
    Q1, r1 = tot[3, :K], tot[3, K]
    d0 = n0 * (XD - 1)
    d1 = n1 * (XD - 1)
    cmi0 = P0 / d0
    cmi1 = P1 / d1
    dr = 0.5 * ((XD - 1) * cmi0 * (n0 - r0) + Q0) / d0 \
       + 0.5 * ((XD - 1) * cmi1 * (n1 - r1) + Q1) / d1
    cmi_dims = (np.abs(cmi0 + cmi1) / 2.0).astype(np.float32)
    drs = np.abs(dr).astype(np.float32)
    return cmi_dims, drs


def _param_key(inputs, spec):
    import hashlib
    hsh = hashlib.sha256()
    for k in sorted(inputs):
        if k in ('x_samples', 'y_samples'):
            continue
        hsh.update(k.encode())
        hsh.update(np.ascontiguousarray(inputs[k]).tobytes())
    hsh.update(repr(spec).encode())
    return hsh.hexdigest()


def kernel(**inputs):
    from concourse.bass_utils import run_bass_kernel_spmd

    fc = _fold_consts(inputs)
    spec = _specialize(fc, np.asarray(inputs['x_samples']))
    lay = _const_layout(fc, spec)
    key = _param_key(inputs, spec)
    if key not in _prog_cache:
        _prog_cache[key] = _build_program(fc, spec, lay)
    nc = _prog_cache[key]

    in_maps = _host_inputs(inputs, fc, spec, lay)
    res = run_bass_kernel_spmd(nc, in_maps, core_ids=list(range(NCORES)))
    parts = [r['out'] for r in res.results]
    return _combine(parts)


# revision 18
# speedup vs baseline: 1.4079x; 1.0478x over previous
"""Trainium2 Bass kernel for nn_DR_CML (data-parallel over batch, 8 cores).

Math: xm[b,i,j] = x[b,i]*lm_w[j] + lm_b[j] means every row of `loo` is a
linear function of the scalar s[b,i] = xbar[b] - x[b,i]/xd.  The tiny
H=7 MLPs applied to loo collapse to scalar piecewise-linear functions of
s, and sum_i over the [B,K,xd-1] diff tensor collapses to a quadratic in
y with per-row coefficients:
    S[b,k] = (xd-1)*positive[b,k] + 0.5*(A[b] - 2*B[b]*y + C[b]*y^2 + D[b])
with A = sum_i e^{-lvn} mun^2, B = sum_i e^{-lvn} mun, C = sum_i e^{-lvn},
D = sum_i lvn (i < xd-1).

Per-call exact specialization: the s range is tiny (xbar ~ N(0,1/xd)), so
most relu units are affine-or-zero over the actual data; they fold into
one exact linear term.  Only units whose breakpoint the data straddles
are emitted as ACT relus.

Layout: x is repacked [2*(B/8), xd/2] = [128, 256] so every big op uses
all 128 partitions; per-row sums are halved per partition and pair-summed
with one PE matmul against a constant M (M[k,m]=1 iff k==m or k==m^64).
Each core reduces its 64 rows to a [4,33] tile of masked partial sums
(one PE matmul); the host sums 8 tiles and applies the final formula.
"""
import math

import numpy as np

B, XD, K, H = 512, 512, 32, 7
NCORES = 8
BL = B // NCORES          # 64 rows per core
HC = XD // 2              # 256 columns after repack
N1 = XD - 1

_prog_cache = {}


def _fold_consts(p):
    """Fold linear_map + MLP weights into scalar-MLP coefficients (f64)."""
    lm_w = p['lm_w'].astype(np.float64)
    lm_b = p['lm_b'].astype(np.float64)
    c = lm_b * (XD - 1) / XD

    def fold(w1, b1):
        u = lm_w @ w1.astype(np.float64)
        v_base = lm_b @ w1.astype(np.float64) + b1.astype(np.float64)
        v_c = c @ w1.astype(np.float64) + b1.astype(np.float64)
        return u, v_base, v_c

    u_mu, vb_mu, vc_mu = fold(p['mu_w1'], p['mu_b1'])
    u_lv, vb_lv, vc_lv = fold(p['lv_w1'], p['lv_b1'])
    u_mun, _, vc_mun = fold(p['mun_w1'], p['mun_b1'])
    u_lvn, _, vc_lvn = fold(p['lvn_w1'], p['lvn_b1'])

    return {
        'u_mu': u_mu, 'vb_mu': vb_mu, 'vc_mu': vc_mu,
        'u_lv': u_lv, 'vb_lv': vb_lv, 'vc_lv': vc_lv,
        'u_mun': u_mun, 'vc_mun': vc_mun,
        'u_lvn': u_lvn, 'vc_lvn': vc_lvn,
        'w2_mu': p['mu_w2'][:, 0].astype(np.float64),
        'w2_lv': p['lv_w2'][:, 0].astype(np.float64),
        'w2_mun': p['mun_w2'][:, 0].astype(np.float64),
        'w2_lvn': p['lvn_w2'][:, 0].astype(np.float64),
        'b2_mu': float(p['mu_b2'][0]), 'b2_lv': float(p['lv_b2'][0]),
        'b2_mun': float(p['mun_b2'][0]), 'b2_lvn': float(p['lvn_b2'][0]),
        'ps_b': float(p['ps_b'][0]),
    }


def _specialize(fc, x):
    """Exact per-call relu pruning over the data's s range (i <= xd-2).

    Returns {'mun': (alpha, beta, active), 'lvn': (...)} where active is a
    list of (a, c, sgn) relu units (|w2| folded in) and alpha includes the
    relevant output bias (b2_mun / b2_lvn).
    """
    x64 = x.astype(np.float64)
    xbar = x64.mean(1)
    s = xbar[:, None] - x64[:, :N1] / XD
    smin, smax = float(s.min()), float(s.max())

    out = {}
    for name, u_all, v_all, w2_all, b2 in (
            ('mun', fc['u_mun'], fc['vc_mun'], fc['w2_mun'], fc['b2_mun']),
            ('lvn', fc['u_lvn'], fc['vc_lvn'], fc['w2_lvn'], fc['b2_lvn'])):
        alpha, beta = b2, 0.0
        active = []
        for u, v, w2 in zip(u_all, v_all, w2_all):
            if w2 == 0.0:
                continue
            lo = min(u * smin, u * smax) + v
            hi = max(u * smin, u * smax) + v
            if lo >= 0.0:          # linear over the data
                alpha += w2 * v
                beta += w2 * u
            elif hi <= 0.0:        # identically zero over the data
                pass
            else:                  # genuinely piecewise on the data
                active.append((float(abs(w2) * u), float(abs(w2) * v),
                               1.0 if w2 > 0 else -1.0))
        out[name] = (float(alpha), float(beta), active)
    return out


def _const_layout(fc, spec):
    """Column layout of the packed consts tensor + bias-value table."""
    bias_vals = []
    for _, _, act in (spec['mun'], spec['lvn']):
        for a, c, sgn in act:
            bias_vals.append(c)
    bias_vals += [fc['b2_mu'], fc['b2_lv'], -math.log(2.0), -fc['ps_b']]
    seen, ordered = set(), []
    for v in bias_vals:
        if v not in seen:
            seen.add(v)
            ordered.append(v)
    nb = len(ordered)
    lay = {
        'bias_vals': ordered,
        'bias0': 128,                 # after M
        'posa': 128 + nb,
        'posc': 128 + nb + 1,
        'w2sel': 128 + nb + 2,        # 4 cols
        'width': 128 + nb + 6,
    }
    return lay


def _build_program(fc, spec, lay):
    from contextlib import ExitStack
    import concourse.tile as tile
    from concourse import bacc, mybir

    f32 = mybir.dt.float32
    bf16 = mybir.dt.bfloat16
    Alu = mybir.AluOpType
    Act = mybir.ActivationFunctionType

    nc = bacc.Bacc("TRN2", target_bir_lowering=False, debug=False,
                   num_devices=NCORES)

    xt_d = nc.dram_tensor("xt", [2 * BL, HC], f32, kind="ExternalInput").ap()
    yv_d = nc.dram_tensor("yv", [BL, K], f32, kind="ExternalInput").ap()
    psw_d = nc.dram_tensor("psw", [2 * BL, HC], f32, kind="ExternalInput").ap()
    tc_d = nc.dram_tensor("consts", [128, lay['width']], f32,
                          kind="ExternalInput").ap()
    out_d = nc.dram_tensor("out", [4, K + 1], f32, kind="ExternalOutput").ap()

    bias_idx = {v: lay['bias0'] + i for i, v in enumerate(lay['bias_vals'])}
    LN2 = math.log(2.0)
    a_mun, b_mun, act_mun = spec['mun']
    a_lvn, b_lvn, act_lvn = spec['lvn']

    with tile.TileContext(nc) as tcx, ExitStack() as ctx:
        sb = ctx.enter_context(tcx.tile_pool(name="sb", bufs=1))
        ps = ctx.enter_context(tcx.tile_pool(name="ps", bufs=1, space="PSUM"))

        # ---- DMAs: x + consts on sync; y + psw on gpsimd queue
        tx = sb.tile([128, HC], f32, tag="tx")
        nc.sync.dma_start(tx[:], xt_d)
        tc = sb.tile([128, lay['width']], f32, tag="tc")
        nc.sync.dma_start(tc[:], tc_d)
        ty = sb.tile([BL, K], f32, tag="ty")
        nc.gpsimd.dma_start(ty[:], yv_d)
        tpsw = sb.tile([128, HC], f32, tag="tpsw")
        nc.gpsimd.dma_start(tpsw[:], psw_d)

        warm = sb.tile([1, 1], f32, tag="warm")
        nc.scalar.activation(warm[:], nc.const_aps.tensor(0.0, (1, 1)),
                             Act.Exp, bias=0.0, scale=1.0)

        def bc(val, p0=0, p1=128):
            j = bias_idx[val]
            return tc[p0:p1, j:j + 1]

        M = tc[:, 0:128]
        Mhalf = tc[:, 0:BL]
        iden = tc[0:BL, 0:BL]

        # ---- xbar, s (f32, [128, 256])
        xsum = sb.tile([128, 1], f32, tag="xsum")
        nc.vector.tensor_reduce(xsum[:], tx[:], mybir.AxisListType.X, Alu.add)
        xbs = ps.tile([128, 1], f32, tag="xbs")
        nc.tensor.matmul(xbs[:], M, xsum[:], start=True, stop=True)
        xbar = sb.tile([128, 1], f32, tag="xbar")
        nc.vector.tensor_scalar(xbar[:], xbs[:], 1.0 / XD, None, Alu.mult)
        s = sb.tile([128, HC], f32, tag="s")
        nc.vector.tensor_scalar(s[:], tx[:], -1.0 / XD, xbar[:], Alu.mult, Alu.add)

        # ---- mun tile: affine + active relus (bf16)
        def build_pwl(alpha, beta, active, tag):
            aff = sb.tile([128, HC], f32, tag=f"{tag}_aff")
            nc.vector.tensor_scalar(aff[:], s[:], beta, alpha, Alu.mult, Alu.add)
            cur = aff
            for idx, (a, c, sgn) in enumerate(active):
                t = sb.tile([128, HC], f32, tag=f"{tag}_u{idx}")
                nc.scalar.activation(t[:], s[:], Act.Relu, bias=bc(c),
                                     scale=float(a))
                nxt = sb.tile([128, HC], f32, tag=f"{tag}_c{idx}")
                nc.vector.tensor_tensor(nxt[:], cur[:], t[:],
                                        Alu.add if sgn > 0 else Alu.subtract)
                cur = nxt
            return cur

        mun_t = build_pwl(a_mun, b_mun, act_mun, "mn")
        lva = build_pwl(a_lvn, b_lvn, act_lvn, "lv")

        # ---- lvn = tanh(lva), ev = exp(-lvn); zero the (i=xd-1) slot, then
        #      accumulate A,B,C,D into columns of one acc tile.
        lvn = sb.tile([128, HC], f32, tag="lvn")
        nc.scalar.activation(lvn[:], lva[:], Act.Tanh, bias=0.0, scale=1.0)
        ev = sb.tile([128, HC], f32, tag="ev")
        exp_inst = nc.scalar.activation(ev[:], lvn[:], Act.Exp, bias=0.0,
                                        scale=-1.0)

        acc = sb.tile([128, 4], f32, tag="acc")   # cols: A,B,C,D
        nc.vector.memset(lvn[BL:128, HC - 1:HC], 0.0)
        nc.vector.tensor_reduce(acc[:, 3:4], lvn[:], mybir.AxisListType.X,
                                Alu.add)
        nc.vector.memset(ev[BL:128, HC - 1:HC], 0.0)
        nc.vector.tensor_reduce(acc[:, 2:3], ev[:], mybir.AxisListType.X,
                                Alu.add)
        em = sb.tile([128, HC], f32, tag="em")
        nc.vector.scalar_tensor_tensor(em[:], ev[:], 1.0, mun_t[:],
                                       Alu.mult, Alu.mult,
                                       accum_out=acc[:, 1:2])
        emm = sb.tile([128, HC], f32, tag="emm")
        nc.vector.scalar_tensor_tensor(emm[:], em[:], 1.0, mun_t[:],
                                       Alu.mult, Alu.mult,
                                       accum_out=acc[:, 0:1])
        # ---- staging: s[.,xd-2], x[.,xd-1] (rows 64:128, top zeroed) and
        #      the propensity dot zp (pair-summed by the same mm)
        stg = sb.tile([128, 3], f32, tag="stg")
        nc.gpsimd.memset(stg[0:BL, 0:2], 0.0)
        nc.gpsimd.tensor_copy(stg[BL:128, 0:1], s[BL:128, HC - 2:HC - 1])
        nc.gpsimd.tensor_copy(stg[BL:128, 1:2], tx[BL:128, HC - 1:HC])
        junk = sb.tile([128, HC], f32, tag="junk")
        nc.vector.scalar_tensor_tensor(junk[:], tx[:], 1.0, tpsw[:],
                                       Alu.mult, Alu.mult,
                                       accum_out=stg[:, 2:3])
        sel = ps.tile([BL, 3], f32, tag="sel")
        nc.tensor.matmul(sel[:], Mhalf, stg[:], start=True, stop=True)
        selsb = sb.tile([BL, 3], f32, tag="selsb")
        nc.vector.tensor_copy(selsb[:], sel[:])

        accp = ps.tile([BL, 4], f32, tag="accp")
        nc.tensor.matmul(accp[:], Mhalf, acc[:], start=True, stop=True)
        accs = sb.tile([BL, 4], f32, tag="accs")
        nc.vector.tensor_copy(accs[:], accp[:])

        # ---- positive branch: mu0/lv0 (xbar), mu1/lv1 (s[.,xd-2])
        xz = sb.tile([BL, 4 * H], f32, tag="xz")
        xb_b = xbar[0:BL, 0:1].broadcast_to([BL, H])
        sl_b = selsb[:, 0:1].broadcast_to([BL, H])
        nc.gpsimd.tensor_copy(xz[:, 0:H], xb_b)
        nc.gpsimd.tensor_copy(xz[:, H:2 * H], sl_b)
        nc.gpsimd.tensor_copy(xz[:, 2 * H:3 * H], xb_b)
        nc.gpsimd.tensor_copy(xz[:, 3 * H:4 * H], sl_b)
        zt = ps.tile([4 * H, BL], f32, tag="zt")
        nc.tensor.transpose(zt[:], xz[:], iden)
        hpos = sb.tile([4 * H, BL], f32, tag="hpos")
        hpos_inst = nc.scalar.activation(hpos[:], zt[:], Act.Relu,
                             bias=tc[0:4 * H, lay['posc']:lay['posc'] + 1],
                             scale=tc[0:4 * H, lay['posa']:lay['posa'] + 1])
        from concourse.tile import add_dep_helper
        add_dep_helper(hpos_inst.ins, exp_inst.ins, sync=True,
                       reason="keep ScalarE on the lvn critical chain")
        mlvp = ps.tile([BL, 4], f32, tag="mlvp")
        nc.tensor.matmul(mlvp[:], hpos[:],
                         tc[0:4 * H, lay['w2sel']:lay['w2sel'] + 4],
                         start=True, stop=True)
        mlv = sb.tile([BL, 4], f32, tag="mlv")
        nc.scalar.activation(mlv[:, 0:2], mlvp[:, 0:2], Act.Identity,
                             bias=bc(fc['b2_mu'], 0, BL), scale=1.0)
        nc.scalar.activation(mlv[:, 2:4], mlvp[:, 2:4], Act.Tanh,
                             bias=bc(fc['b2_lv'], 0, BL), scale=1.0)
        ge2 = sb.tile([BL, 2], f32, tag="ge2")
        nc.scalar.activation(ge2[:], mlv[:, 2:4], Act.Exp,
                             bias=bc(-LN2, 0, BL), scale=-1.0)
        gn2 = sb.tile([BL, 2], f32, tag="gn2")
        nc.gpsimd.tensor_scalar(gn2[:], ge2[:], -1.0, None, Alu.mult)
        kk2 = sb.tile([BL, 2], f32, tag="kk2")
        nc.gpsimd.tensor_scalar(kk2[:], mlv[:, 2:4], -0.5, None, Alu.mult)

        # pos[b,k]: k=0 from (mu0,lv0), k>=1 from (mu1,lv1)
        d1 = sb.tile([BL, K], f32, tag="d1")
        nc.gpsimd.tensor_scalar(d1[:], ty[:], mlv[:, 1:2], None, Alu.subtract)
        d1sq = sb.tile([BL, K], f32, tag="d1sq")
        nc.gpsimd.tensor_tensor(d1sq[:], d1[:], d1[:], Alu.mult)
        pos = sb.tile([BL, K], f32, tag="pos")
        nc.gpsimd.tensor_scalar(pos[:], d1sq[:], gn2[:, 1:2], kk2[:, 1:2],
                                Alu.mult, Alu.add)
        d0 = sb.tile([BL, 1], f32, tag="d0")
        nc.gpsimd.tensor_scalar(d0[:], ty[:, 0:1], mlv[:, 0:1], None,
                                Alu.subtract)
        d0sq = sb.tile([BL, 1], f32, tag="d0sq")
        nc.gpsimd.tensor_tensor(d0sq[:], d0[:], d0[:], Alu.mult)
        nc.gpsimd.tensor_scalar(pos[:, 0:1], d0sq[:], gn2[:, 0:1],
                                kk2[:, 0:1], Alu.mult, Alu.add)

        # ---- propensity weights:  p = 1/(1+e),  e = exp(-(z+ps_b))
        # w1 = (1+e)/(1.0001 + 1e-4 e) ; w0 = (1+e)/(1e-4 + 1.0001 e)
        epr = sb.tile([BL, 1], f32, tag="epr")
        nc.scalar.activation(epr[:], sel[:, 2:3], Act.Exp,
                             bias=bc(-fc['ps_b'], 0, BL), scale=-1.0)
        num = sb.tile([BL, 1], f32, tag="num")
        nc.vector.tensor_scalar(num[:], epr[:], 1.0, None, Alu.add)
        den1 = sb.tile([BL, 1], f32, tag="den1")
        nc.vector.tensor_scalar(den1[:], epr[:], 1e-4, 1.0 + 1e-4,
                                Alu.mult, Alu.add)
        den0 = sb.tile([BL, 1], f32, tag="den0")
        nc.vector.tensor_scalar(den0[:], epr[:], 1.0 + 1e-4, 1e-4,
                                Alu.mult, Alu.add)
        r1 = sb.tile([BL, 1], f32, tag="r1")
        nc.vector.reciprocal(r1[:], den1[:])
        w1v = sb.tile([BL, 1], f32, tag="w1v")
        nc.vector.tensor_tensor(w1v[:], num[:], r1[:], Alu.mult)
        r0 = sb.tile([BL, 1], f32, tag="r0")
        nc.vector.reciprocal(r0[:], den0[:])
        w0v = sb.tile([BL, 1], f32, tag="w0v")
        nc.vector.tensor_tensor(w0v[:], num[:], r0[:], Alu.mult)

        F = sb.tile([BL, 4], f32, tag="F")
        nc.gpsimd.tensor_scalar(F[:, 0:1], selsb[:, 1:2], 0.0, None,
                                Alu.is_equal)
        nc.gpsimd.tensor_scalar(F[:, 2:3], selsb[:, 1:2], 1.0, None,
                                Alu.is_equal)
        nc.gpsimd.tensor_tensor(F[:, 1:2], F[:, 0:1], w0v[:], Alu.mult)
        nc.gpsimd.tensor_tensor(F[:, 3:4], F[:, 2:3], w1v[:], Alu.mult)

        # ---- S = 511*pos + q0 + q1*y + q2*y^2 ; R = [S | 1]
        q2 = sb.tile([BL, 1], f32, tag="q2")
        nc.vector.tensor_scalar(q2[:], accp[:, 2:3], 0.5, None, Alu.mult)
        q1 = sb.tile([BL, 1], f32, tag="q1")
        nc.vector.tensor_scalar(q1[:], accp[:, 1:2], -1.0, None, Alu.mult)
        q0h = sb.tile([BL, 1], f32, tag="q0h")
        nc.vector.tensor_tensor(q0h[:], accs[:, 0:1], accs[:, 3:4], Alu.add)
        q0 = sb.tile([BL, 1], f32, tag="q0")
        nc.vector.tensor_scalar(q0[:], q0h[:], 0.5, None, Alu.mult)

        R = sb.tile([BL, K + 1], f32, tag="R")
        nc.gpsimd.memset(R[:, K:K + 1], 1.0)
        yt2 = sb.tile([BL, K], f32, tag="yt2")
        nc.gpsimd.tensor_tensor(yt2[:], ty[:], ty[:], Alu.mult)
        S1 = sb.tile([BL, K], f32, tag="S1")
        nc.vector.tensor_scalar(S1[:], yt2[:], q2[:], q0[:], Alu.mult, Alu.add)
        S2 = sb.tile([BL, K], f32, tag="S2")
        nc.vector.scalar_tensor_tensor(S2[:], ty[:], q1[:], S1[:],
                                       Alu.mult, Alu.add)
        nc.vector.scalar_tensor_tensor(R[:, 0:K], pos[:], float(XD - 1),
                                       S2[:], Alu.mult, Alu.add)

        P = ps.tile([4, K + 1], f32, tag="P")
        nc.tensor.matmul(P[:], F[:], R[:], start=True, stop=True)
        outs = sb.tile([4, K + 1], f32, tag="outs")
        nc.vector.tensor_copy(outs[:], P[:])
        nc.sync.dma_start(out_d, outs[:])

    nc.compile()
    return nc


def _host_inputs(inputs, fc, spec, lay):
    x = np.ascontiguousarray(inputs['x_samples'], dtype=np.float32)
    y = np.ascontiguousarray(inputs['y_samples'], dtype=np.float32)
    ps_w = inputs['ps_w'].astype(np.float32)[:, 0]

    # psw repack: rows p<64 <- ps_w[0:256]; rows p>=64 <- ps_w[256:511] + 0
    psw2 = np.zeros((2, HC), np.float32)
    psw2[0] = ps_w[0:HC]
    psw2[1, 0:HC - 1] = ps_w[HC:N1]
    psw = np.ascontiguousarray(
        np.broadcast_to(psw2[:, None, :], (2, BL, HC)).reshape(128, HC))

    M = np.zeros((128, 128), np.float32)
    idx = np.arange(128)
    M[idx, idx] = 1.0
    M[idx ^ 64, idx] = 1.0

    consts = np.zeros((128, lay['width']), np.float32)
    consts[:, 0:128] = M
    for i, v in enumerate(lay['bias_vals']):
        consts[:, lay['bias0'] + i] = v
    posa = np.zeros(4 * H); posc = np.zeros(4 * H)
    posa[0:H] = fc['u_mu'];          posc[0:H] = fc['vb_mu']
    posa[H:2 * H] = fc['u_mu'];      posc[H:2 * H] = fc['vc_mu']
    posa[2 * H:3 * H] = fc['u_lv'];  posc[2 * H:3 * H] = fc['vb_lv']
    posa[3 * H:4 * H] = fc['u_lv'];  posc[3 * H:4 * H] = fc['vc_lv']
    consts[0:4 * H, lay['posa']] = posa
    consts[0:4 * H, lay['posc']] = posc
    w2sel = np.zeros((4 * H, 4), np.float32)
    w2sel[0:H, 0] = fc['w2_mu']
    w2sel[H:2 * H, 1] = fc['w2_mu']
    w2sel[2 * H:3 * H, 2] = fc['w2_lv']
    w2sel[3 * H:4 * H, 3] = fc['w2_lv']
    consts[0:4 * H, lay['w2sel']:lay['w2sel'] + 4] = w2sel

    in_maps = []
    for i in range(NCORES):
        xs = x[i * BL:(i + 1) * BL]                       # [64, 512]
        xt = np.ascontiguousarray(
            xs.reshape(BL, 2, HC).transpose(1, 0, 2).reshape(128, HC))
        in_maps.append({
            'xt': xt,
            'yv': y[i * BL:(i + 1) * BL],
            'psw': psw, 'consts': consts,
        })
    return in_maps


def _combine(parts):
    tot = np.zeros((4, K + 1), np.float64)
    for p in parts:
        tot += p.astype(np.float64)
    P0, n0 = tot[0, :K], tot[0, K]
    Q0, r0 = tot[1, :K], tot[1, K]
    P1, n1 = tot[2, :K], tot[2, K]
    Q1, r1 = tot[3, :K], tot[3, K]
    d0 = n0 * (XD - 1)
    d1 = n1 * (XD - 1)
    cmi0 = P0 / d0
    cmi1 = P1 / d1
    dr = 0.5 * ((XD - 1) * cmi0 * (n0 - r0) + Q0) / d0 \
       + 0.5 * ((XD - 1) * cmi1 * (n1 - r1) + Q1) / d1
    cmi_dims = (np.abs(cmi0 + cmi1) / 2.0).astype(np.float32)
    drs = np.abs(dr).astype(np.float32)
    return cmi_dims, drs


def _param_key(inputs, spec):
    import hashlib
    hsh = hashlib.sha256()
    for k in sorted(inputs):
        if k in ('x_samples', 'y_samples'):
            continue
        hsh.update(k.encode())
        hsh.update(np.ascontiguousarray(inputs[k]).tobytes())
    hsh.update(repr(spec).encode())
    return hsh.hexdigest()


def kernel(**inputs):
    from concourse.bass_utils import run_bass_kernel_spmd

    fc = _fold_consts(inputs)
    spec = _specialize(fc, np.asarray(inputs['x_samples']))
    lay = _const_layout(fc, spec)
    key = _param_key(inputs, spec)
    if key not in _prog_cache:
        _prog_cache[key] = _build_program(fc, spec, lay)
    nc = _prog_cache[key]

    in_maps = _host_inputs(inputs, fc, spec, lay)
    res = run_bass_kernel_spmd(nc, in_maps, core_ids=list(range(NCORES)))
    parts = [r['out'] for r in res.results]
    return _combine(parts)
